# revision 46
# baseline (speedup 1.0000x reference)
"""GAT (graph attention) kernel for Trainium2, 8-core SPMD.

Sharding: core c handles heads {2g, 2g+1} (g = c//2) for n-block
[n0, n0+2048) (n0 = (c%2)*2048).  The fp16 mask slice (16.8MB) is loaded
once per core and reused by both heads; all large tensors are fp16.

Per-head math (head k):
    h = x @ W_k.T + b_k                        # (N, F)
    l[n] = h[n].a_left ; r[m] = h[m].a_right   # PE, via host-precomputed
                                               #   W_k.T a vectors
    em[m, n] = exp(leaky(l+r, 0.2))/16 * mask  # hacked ACT Exp table
    out = elu( (h.T @ em) / (1.T @ em) )       # PE fp16 matmuls

Key tricks:
  - The Exp activation table is rewritten so table(x) = e^{leaky(x)}/16
    on both sides (the 1/16 keeps fp16 range and cancels in softmax).
  - Masking is ADDITIVE and pre-activation: host ships
    maskadd = (mask-1)*60 (fp16); one DVE tensor_tensor (2x mode) adds
    the l broadcast; act bias supplies r[m] per partition.  Masked
    entries become e^{0.2(z-60)}/16 ~ 1e-6 -- no post-mask multiply.
  - The ACT hardware special-cases exp(exactly 0) = 1.0, bypassing the
    table.  r is carried in fp32 and scaled by (1+2^-18) so the fp32
    arg pm+r can never cancel to exactly 0; the ELU clamp is min(u,-2e-7)
    instead of min(u,0) for the same reason.
  - Aggregation and sums are fp16 PE matmuls accumulating in PSUM over
    32 m-chunks; sums are split DVE/GPSIMD/PE to balance engines, with
    the partition-reduction matmuls joining the same PSUM groups.
  - agg is staged out of PSUM (scaled 1/1024 into fp16) right after each
    sweep so banks free early; each head's epilogue is deferred until
    after the next head's sweep is emitted, keeping the in-order ACT
    queue free of stalls at head boundaries.
  - 1/sums row is broadcast across partitions with a ones-column PE
    matmul (no DRAM roundtrip).
  - W2 / wlr ride as extra columns of the xT / xn transfers; scratch
    roundtrips use the Activation HWDGE queue (the SP sequencer costs
    565ns per dma_start, so instruction count on it is minimized).

Engine busy per core (cost model): Act ~125us (em creation floor),
DVE ~131us, PE ~120us, DMA ~75us; ~176us per iteration end to end.
"""

import json
import os
import shutil
import tempfile

import numpy as np

import concourse.bass as bass
import concourse.tile as tile
from concourse import bacc, mybir
from concourse.bass_utils import run_bass_kernel_spmd

N_NODES = 4096
F_IN = 512
K_HEADS = 8
F_OUT = 128
NEG_SLOPE = 0.2
N_CORES = 8

HPC = 2          # heads per core
NB = 2048        # n-block per core
B_MASK = 60.0    # additive mask fill (pre-activation)
KSCALE = 1.0 / 16.0  # global scale baked into the act table (cancels in softmax)

f32 = mybir.dt.float32
f16 = mybir.dt.float16

# m-chunk sum assignment: DVE accumulator / GPSIMD accumulator / PE matmul
SUMS_DVE_CHUNKS = frozenset(mc for mc in range(32) if mc % 8 < 3)   # 12
SUMS_GP_CHUNKS = frozenset(mc for mc in range(32) if mc % 16 == 3)  # 2
PREMASK_GP_CHUNKS = frozenset()


# --------------------------------------------------------------------------- #
# activation-table hack: Exp computes e^{leaky_relu(x, 0.2)}/16
# --------------------------------------------------------------------------- #
def _make_hacked_act_dir(dst):
    from neuronxcc.driver.Job import Job
    from neuronxcc.driver.jobs.support.FindActInfo import findActInfoFile

    src = os.path.dirname(findActInfoFile(Job.getPackageDir(), "gen3"))
    os.makedirs(dst, exist_ok=True)
    for fn in os.listdir(src):
        shutil.copy(os.path.join(src, fn), os.path.join(dst, fn))

    info = json.load(open(os.path.join(dst, "act_info.json")))
    for s in info["act_func_sets"]:
        if "exp" not in s["act"]:
            continue
        prof = json.load(open(os.path.join(dst, s["profile_json"])))
        start = prof["func_to_bkt_start_idx"]["exp"]
        starts = sorted(prof["func_to_bkt_start_idx"].values())
        ends = [e for e in starts if e > start]
        end = ends[0] if ends else prof["bkt_entry_cnt"]

        path = os.path.join(dst, s["bkt_bin"])
        b = np.fromfile(path, dtype=np.float32).reshape(-1, 8).copy()
        sl = b[start:end]
        neg = sl[:, 4] < 0.0
        x0 = sl[neg, 4].astype(np.float64)
        g = np.exp(NEG_SLOPE * x0) * KSCALE
        sl[neg, 0] = g
        sl[neg, 1] = NEG_SLOPE * g
        sl[neg, 2] = NEG_SLOPE**2 * g / 2.0
        sl[neg, 3] = NEG_SLOPE**3 * g / 6.0
        # positive side keeps e^x shape, scaled by KSCALE
        sl[~neg, 0:4] *= KSCALE
        b[start:end] = sl
        b.tofile(path)
    return os.path.join(dst, "act_info.json")


_ACT_DIR = None


def setup_act_tables():
    global _ACT_DIR
    if _ACT_DIR is None:
        d = os.path.join(tempfile.gettempdir(), "gat_act_tables_v2")
        _ACT_DIR = _make_hacked_act_dir(d)
    os.environ["BASS_ACT_ROOT_JSON_PATH"] = _ACT_DIR
    return _ACT_DIR


# --------------------------------------------------------------------------- #
# bass program
# --------------------------------------------------------------------------- #
def build(num_devices=N_CORES, timing_mode=False, repeat=1, debug_taps=False):
    setup_act_tables()

    n = N_NODES
    nb = NB
    cseg = F_IN // 128   # 4 contraction chunks
    mc_cnt = n // 128    # 32 m-chunks
    nseg = nb // 512     # 4 PSUM segments per n-block

    nc = bacc.Bacc("TRN2", target_bir_lowering=False, debug=False, num_devices=num_devices)

    big_kind = "Internal" if timing_mode else "ExternalInput"
    # x.T with W2 columns appended; xn slice with wlr columns appended —
    # fewer big DMAs keeps the SP sequencer (565ns per dma_start) off the
    # critical path.
    xT_d = nc.dram_tensor("xT", [F_IN, n + HPC * F_OUT], f16, kind=big_kind).ap()
    xn_d = nc.dram_tensor("xn", [F_IN, nb + 4], f16, kind=big_kind).ap()
    maskT_d = nc.dram_tensor("maskaddT", [n, nb], f16, kind=big_kind).ap()
    b2_d = nc.dram_tensor("b2", [1, HPC * F_OUT], f16, kind="ExternalInput").ap()
    crv_d = nc.dram_tensor("crv", [2, 1], f32, kind="ExternalInput").ap()
    out_kind = "Internal" if timing_mode else "ExternalOutput"
    out_d = nc.dram_tensor("out2", [HPC * F_OUT, nb], f16, kind=out_kind).ap()
    sink_d = None
    if timing_mode:
        sink_d = nc.dram_tensor("sink", [1, 128], f32, kind="ExternalOutput").ap()
    dbg = {}
    if debug_taps:
        dbg["lbc"] = nc.dram_tensor("dbg_lbc", [128, HPC * nb], f16, kind="ExternalOutput").ap()
        dbg["rsc"] = nc.dram_tensor("dbg_rsc", [128, HPC * 32], f16, kind="ExternalOutput").ap()
        dbg["hmf"] = nc.dram_tensor("dbg_hmf", [128, HPC * 32 * F_OUT], f16, kind="ExternalOutput").ap()
        dbg["em0"] = nc.dram_tensor("dbg_em0", [128, HPC * nb], f16, kind="ExternalOutput").ap()
        dbg["stage"] = nc.dram_tensor("dbg_stage", [128, HPC * nb], f16, kind="ExternalOutput").ap()
        dbg["rs1"] = nc.dram_tensor("dbg_rs1", [1, HPC * nb], f16, kind="ExternalOutput").ap()

    lr_dram = nc.dram_tensor("lr_scratch", [2, NB], f16, kind="Internal")   # l, row=head
    r32_dram = nc.dram_tensor("r32_scratch", [2, N_NODES], f32, kind="Internal")  # r, row=head

    def dram_ap(handle, offset, pattern):
        return bass.AP(tensor=handle.ap().tensor, offset=offset, ap=pattern)

    with tile.TileContext(nc) as tc:
        with tc.tile_pool(name="consts", bufs=1) as consts:
            if timing_mode:
                fz = consts.tile([128, nb + 4], f16, tag="fz")
                nc.vector.memset(fz, 0.0)
                for c in range(cseg):
                    for q in range(n // nb):
                        nc.sync.dma_start(
                            out=xT_d[c * 128 : (c + 1) * 128, q * nb : (q + 1) * nb],
                            in_=fz[:, :nb],
                        )
                    nc.sync.dma_start(
                        out=xT_d[c * 128 : (c + 1) * 128, n : n + HPC * F_OUT],
                        in_=fz[:, : HPC * F_OUT],
                    )
                for c in range(cseg):
                    nc.sync.dma_start(
                        out=xn_d[c * 128 : (c + 1) * 128, :], in_=fz[:, : nb + 4]
                    )
                for r in range(mc_cnt):
                    nc.sync.dma_start(out=maskT_d[r * 128 : (r + 1) * 128, :], in_=fz[:, :nb])

            last_out = [None]
            for _rep in range(repeat):
                # ------------- constants ------------- #
                b2_sb = consts.tile([1, HPC * F_OUT], f16, tag="b2")
                nc.sync.dma_start(out=b2_sb, in_=b2_d)
                crv_sb = consts.tile([2, 1], f32, tag="crv")
                nc.sync.dma_start(out=crv_sb, in_=crv_d)
                ones_sb = consts.tile([128, 1], f16, tag="ones")
                nc.vector.memset(ones_sb, 1.0)
                onesrow = consts.tile([65, 128], f16, tag="onesrow")
                nc.vector.memset(onesrow, 1.0)

                h_mf = consts.tile([128, HPC, mc_cnt, F_OUT], f16, tag="h_mf")
                l_bc = consts.tile([128, HPC, nb], f16, tag="l_bc")
                r_sc = consts.tile([128, HPC, mc_cnt], f32, tag="r_sc")

                # ------------- pre-phase: projections ------------- #
                # xT loads are column-grouped (1024 cols) so r, h_mf and the
                # r_sc readbacks complete incrementally; small scratch
                # roundtrips ride the Activation HWDGE queue to stay off the
                # streaming (SP) queue.
                with (
                    tc.tile_pool(name="pre", bufs=1) as pre,
                    tc.tile_pool(name="prePS", bufs=2, space="PSUM") as prePS,
                ):
                    xn_sb = pre.tile([128, cseg, nb + 4], f16, tag="xn")
                    for c in range(cseg):
                        nc.sync.dma_start(out=xn_sb[:, c, :], in_=xn_d[c * 128 : (c + 1) * 128, :])
                    xT_sb = pre.tile([128, cseg, n + HPC * F_OUT], f16, tag="xT")
                    # column-grouped loads: r segments (and their readbacks)
                    # complete before the full x transfer, shortening the
                    # first-activation chain at iteration boundaries
                    for g in range(4):
                        for c in range(cseg):
                            nc.sync.dma_start(
                                out=xT_sb[:, c, g * 1024 : (g + 1) * 1024],
                                in_=xT_d[c * 128 : (c + 1) * 128, g * 1024 : (g + 1) * 1024],
                            )
                    for c in range(cseg):
                        nc.sync.dma_start(
                            out=xT_sb[:, c, n : n + HPC * F_OUT],
                            in_=xT_d[c * 128 : (c + 1) * 128, n : n + HPC * F_OUT],
                        )

                    # l = xn.T @ wl (+b.al via crv) ; r = xT.T @ wr
                    # partition = head.  r stays fp32, nudged off the fp16
                    # grid so pm + r can never be exactly 0 (the ACT hardware
                    # special-cases exp(0) = 1, bypassing the hacked table).
                    lr_sb = pre.tile([2, nseg, 512], f16, tag="lr_sb")
                    lrr_sb = pre.tile([2, 8, 512], f32, tag="lrr_sb")
                    for j in range(nseg):
                        lr2 = prePS.tile([2, 512], f32, tag="lr2")
                        for c in range(cseg):
                            nc.tensor.matmul(
                                lr2,
                                lhsT=xn_sb[:, c, nb : nb + 2],
                                rhs=xn_sb[:, c, j * 512 : (j + 1) * 512],
                                start=(c == 0),
                                stop=(c == cseg - 1),
                            )
                        nc.vector.tensor_copy(out=lr_sb[:, j, :], in_=lr2)
                    for h in range(HPC):
                        nc.scalar.dma_start(
                            out=dram_ap(lr_dram, h * nb, [[1, nb]]),
                            in_=lr_sb[h : h + 1, :, :],
                        )
                        nc.scalar.dma_start(
                            out=l_bc[:, h, :],
                            in_=dram_ap(lr_dram, h * nb, [[0, 128], [1, nb]]),
                        )

                    for g in range(4):
                        for j in (2 * g, 2 * g + 1):
                            lr2 = prePS.tile([2, 512], f32, tag="lr2")
                            for c in range(cseg):
                                nc.tensor.matmul(
                                    lr2,
                                    lhsT=xn_sb[:, c, nb + 2 : nb + 4],
                                    rhs=xT_sb[:, c, j * 512 : (j + 1) * 512],
                                    start=(c == 0),
                                    stop=(c == cseg - 1),
                                )
                            # r gets + (b.a_left + b.a_right) folded in
                            nc.vector.tensor_scalar(
                                out=lrr_sb[:, j, :],
                                in0=lr2,
                                scalar1=1.0 + 2.0**-18,
                                scalar2=crv_sb,
                                op0=mybir.AluOpType.mult,
                                op1=mybir.AluOpType.add,
                            )
                        for h in range(HPC):
                            nc.scalar.dma_start(
                                out=dram_ap(r32_dram, h * n + 1024 * g, [[1, 1024]]),
                                in_=lrr_sb[h : h + 1, 2 * g : 2 * g + 2, :],
                            )
                            nc.scalar.dma_start(
                                out=r_sc[:, h, 8 * g : 8 * g + 8],
                                in_=dram_ap(
                                    r32_dram, h * n + 1024 * g, [[1, 128], [128, 8]]
                                ),
                            )

                    # h_mf[m, f] for both heads: lhsT = xT chunk, rhs = W2
                    for mc in range(mc_cnt):
                        hmf_ps = prePS.tile([128, HPC * F_OUT], f32, tag="hmf")
                        for c in range(cseg):
                            nc.tensor.matmul(
                                hmf_ps,
                                lhsT=xT_sb[:, c, mc * 128 : (mc + 1) * 128],
                                rhs=xT_sb[:, c, n : n + HPC * F_OUT],
                                start=(c == 0),
                                stop=False,
                            )
                        nc.tensor.matmul(
                            hmf_ps, lhsT=onesrow[0:1, :], rhs=b2_sb, start=False, stop=True
                        )
                        nc.vector.tensor_copy(out=h_mf[:, :, mc, :], in_=hmf_ps)

                # ------------- main: em creation + aggregation ------------- #
                with (
                    tc.tile_pool(name="maskpool", bufs=1) as maskpool,
                    tc.tile_pool(name="work", bufs=3) as work,
                    tc.tile_pool(name="epi", bufs=1) as epi,
                    tc.tile_pool(name="mainPS", bufs=1, space="PSUM") as mainPS,
                    tc.tile_pool(name="rsPS", bufs=1, space="PSUM") as rsPS,
                ):
                    mask_sb = maskpool.tile([128, mc_cnt, nb], f16, tag="mask")
                    if debug_taps:
                        nc.sync.dma_start(out=dbg["lbc"], in_=l_bc[:, :, :])
                        nc.sync.dma_start(out=dbg["rsc"], in_=r_sc[:, :, :])
                        nc.sync.dma_start(out=dbg["hmf"], in_=h_mf[:, :, :, :])

                    def sweep(h):
                        """One head's em sweep.  Aggregation lands in PSUM;
                        it is staged to SBUF (scaled 1/1024, fp16) right away
                        so the banks free without waiting on the epilogue."""
                        agg_ps = []
                        for j in range(nseg):
                            agg_seg = mainPS.tile([128, 512], f32, tag=f"agg{j}")
                            agg_ps.append(agg_seg)
                        sums_psA = mainPS.tile([65, 512], f32, tag="sumsA")
                        sums_psB = mainPS.tile([65, 512], f32, tag="sumsB")

                        def sums_slot(j):
                            # matmul out base partition must be 0/32/64
                            t = sums_psA if j < 2 else sums_psB
                            p = 64 * (j % 2)
                            return t[p : p + 1, :]

                        S_sb = epi.tile([128, nb], f16, tag="S")
                        Sg_sb = epi.tile([128, nb], f16, tag="Sg")
                        nc.gpsimd.memset(S_sb, 0.0)
                        nc.gpsimd.memset(Sg_sb, 0.0)
                        pe_chunks = [
                            mc
                            for mc in range(mc_cnt)
                            if mc not in SUMS_DVE_CHUNKS and mc not in SUMS_GP_CHUNKS
                        ]

                        for mc in range(mc_cnt):
                            msl = mask_sb[:, mc, :]
                            if h == 0:
                                nc.sync.dma_start(
                                    out=msl,
                                    in_=maskT_d[mc * 128 : (mc + 1) * 128, :],
                                )
                            pm = work.tile([128, nb], f16, tag="pm")
                            if mc in PREMASK_GP_CHUNKS:
                                nc.gpsimd.tensor_add(pm, msl, l_bc[:, h, :])
                            else:
                                nc.vector.tensor_tensor(
                                    out=pm,
                                    in0=msl,
                                    in1=l_bc[:, h, :],
                                    op=mybir.AluOpType.add,
                                )
                            em = work.tile([128, nb], f16, tag="em")
                            nc.scalar.activation(
                                out=em,
                                in_=pm,
                                func=mybir.ActivationFunctionType.Exp,
                                bias=r_sc[:, h, mc : mc + 1],
                                scale=1.0,
                            )
                            if debug_taps and mc == 0:
                                nc.sync.dma_start(
                                    out=dbg["em0"][:, h * nb : (h + 1) * nb], in_=em
                                )
                            for j in range(nseg):
                                nc.tensor.matmul(
                                    agg_ps[j],
                                    lhsT=h_mf[:, h, mc, :],
                                    rhs=em[:, j * 512 : (j + 1) * 512],
                                    start=(mc == 0),
                                    stop=(mc == mc_cnt - 1),
                                )
                            if mc in SUMS_DVE_CHUNKS:
                                with nc.allow_low_precision(
                                    reason="fp16 partial-sum accumulator; "
                                    "positive terms, ~32 adds"
                                ):
                                    nc.vector.tensor_tensor(
                                        out=S_sb, in0=S_sb, in1=em, op=mybir.AluOpType.add
                                    )
                            elif mc in SUMS_GP_CHUNKS:
                                nc.gpsimd.tensor_add(Sg_sb, Sg_sb, em)
                            else:
                                first = mc == pe_chunks[0]
                                for j in range(nseg):
                                    nc.tensor.matmul(
                                        sums_slot(j),
                                        lhsT=ones_sb,
                                        rhs=em[:, j * 512 : (j + 1) * 512],
                                        start=first,
                                        stop=False,
                                    )

                        # S/Sg partition-reduces join the same PSUM groups
                        for j in range(nseg):
                            nc.tensor.matmul(
                                sums_slot(j),
                                lhsT=ones_sb,
                                rhs=S_sb[:, j * 512 : (j + 1) * 512],
                                start=False,
                                stop=False,
                            )
                        for j in range(nseg):
                            nc.tensor.matmul(
                                sums_slot(j),
                                lhsT=ones_sb,
                                rhs=Sg_sb[:, j * 512 : (j + 1) * 512],
                                start=False,
                                stop=True,
                            )

                        # stage agg out of PSUM (scaled so it fits fp16) and
                        # take reciprocals now; banks free without waiting on
                        # the rest of the epilogue.
                        stage = epi.tile([128, nb], f16, tag=f"stage{h}")
                        rs2 = epi.tile([65, nb], f16, tag="rs1")
                        rs1 = rs2[64 * h : 64 * h + 1, :]
                        with nc.allow_low_precision(
                            reason="staged agg/1024 and 1/sums in fp16; "
                            "~1e-3 relative, within tolerance"
                        ):
                            for j in range(nseg):
                                nc.vector.tensor_scalar(
                                    out=stage[:, j * 512 : (j + 1) * 512],
                                    in0=agg_ps[j],
                                    scalar1=1.0 / 1024.0,
                                    scalar2=None,
                                    op0=mybir.AluOpType.mult,
                                )
                            for j in range(nseg):
                                nc.vector.reciprocal(
                                    out=rs1[:, j * 512 : (j + 1) * 512],
                                    in_=sums_slot(j),
                                )
                        if debug_taps:
                            nc.sync.dma_start(
                                out=dbg["stage"][:, h * nb : (h + 1) * nb], in_=stage
                            )
                            nc.sync.dma_start(
                                out=dbg["rs1"][:, h * nb : (h + 1) * nb], in_=rs1
                            )
                        return stage, rs1

                    def epilogue(h, stage, rs1):
                        """Deferred: u = stage * bc(rs1) * 1024; out = elu.
                        u overwrites stage in place to save SBUF."""
                        u_sb = stage
                        for j in range(nseg):
                            rs_ps = rsPS.tile([128, 512], f32, tag="rs_ps")
                            nc.tensor.matmul(
                                rs_ps,
                                lhsT=onesrow[64 * h : 64 * h + 1, :],
                                rhs=rs1[:, j * 512 : (j + 1) * 512],
                                start=True,
                                stop=True,
                            )
                            nc.vector.tensor_tensor(
                                out=u_sb[:, j * 512 : (j + 1) * 512],
                                in0=stage[:, j * 512 : (j + 1) * 512],
                                in1=rs_ps,
                                op=mybir.AluOpType.mult,
                            )
                        t_sb = epi.tile([128, nb], f16, tag="t")
                        # clamp to a tiny negative (not 0): exp(exactly 0)
                        # takes a hardware fast path that ignores the table
                        nc.vector.tensor_scalar(
                            out=t_sb,
                            in0=u_sb,
                            scalar1=-2e-7,
                            scalar2=None,
                            op0=mybir.AluOpType.min,
                        )
                        # e^{min(u,0)}: u is carried at 1/1024 scale, so the
                        # table's x5 leak slope needs scale=5*1024
                        nc.scalar.activation(
                            out=t_sb,
                            in_=t_sb,
                            func=mybir.ActivationFunctionType.Exp,
                            scale=5.0 * 1024.0,
                        )
                        # elu = max(u, t/KSCALE - 1), in place on t
                        nc.vector.tensor_scalar(
                            out=t_sb,
                            in0=t_sb,
                            scalar1=1.0 / KSCALE,
                            scalar2=-1.0,
                            op0=mybir.AluOpType.mult,
                            op1=mybir.AluOpType.add,
                        )
                        nc.vector.tensor_scalar(
                            out=u_sb,
                            in0=u_sb,
                            scalar1=1024.0,
                            scalar2=None,
                            op0=mybir.AluOpType.mult,
                        )
                        nc.vector.tensor_tensor(
                            out=t_sb, in0=t_sb, in1=u_sb, op=mybir.AluOpType.max
                        )
                        nc.scalar.dma_start(
                            out=out_d[h * F_OUT : (h + 1) * F_OUT, :], in_=t_sb
                        )
                        last_out[0] = t_sb

                    staged = sweep(0)
                    staged1 = sweep(1)
                    epilogue(0, *staged)
                    epilogue(1, *staged1)

            if timing_mode and sink_d is not None:
                sk = consts.tile([1, 128], f32, tag="sink")
                nc.vector.tensor_copy(out=sk, in_=last_out[0][0:1, 0:128])
                nc.sync.dma_start(out=sink_d, in_=sk)

    nc.compile()
    return nc


# --------------------------------------------------------------------------- #
# host entry point
# --------------------------------------------------------------------------- #
_NC_CACHE = {}


def _get_nc():
    key = (N_NODES, NB)
    if key not in _NC_CACHE:
        _NC_CACHE[key] = build(N_CORES)
    return _NC_CACHE[key]


def make_in_maps(x, mask, W, b, a_left, a_right):
    xT = np.ascontiguousarray(x.T).astype(np.float16)
    maskadd = ((mask.T.astype(np.float32)) - 1.0) * B_MASK
    maskadd = maskadd.astype(np.float16)

    in_maps = []
    for c in range(N_CORES):
        g = c // 2
        nbi = c % 2
        heads = [2 * g, 2 * g + 1]
        Wk = [W[k * F_OUT : (k + 1) * F_OUT, :] for k in heads]
        w2 = np.concatenate([Wk[0].T, Wk[1].T], axis=1).astype(np.float16)
        wlr = np.stack(
            [
                Wk[0].T @ a_left[heads[0]],
                Wk[1].T @ a_left[heads[1]],
                Wk[0].T @ a_right[heads[0]],
                Wk[1].T @ a_right[heads[1]],
            ],
            axis=1,
        ).astype(np.float16)
        b2 = np.concatenate(
            [b[heads[0] * F_OUT : (heads[0] + 1) * F_OUT], b[heads[1] * F_OUT : (heads[1] + 1) * F_OUT]]
        ).reshape(1, -1).astype(np.float16)
        crv = np.zeros((2, 1), np.float32)
        for k in range(2):
            kk = heads[k]
            crv[k, 0] = float(
                b[kk * F_OUT : (kk + 1) * F_OUT] @ a_left[kk]
                + b[kk * F_OUT : (kk + 1) * F_OUT] @ a_right[kk]
            )
        in_maps.append(
            {
                "xT": np.ascontiguousarray(np.concatenate([xT, w2], axis=1)),
                "xn": np.ascontiguousarray(
                    np.concatenate([xT[:, nbi * NB : (nbi + 1) * NB], wlr], axis=1)
                ),
                "maskaddT": np.ascontiguousarray(maskadd[:, nbi * NB : (nbi + 1) * NB]),
                "b2": np.ascontiguousarray(b2),
                "crv": crv,
            }
        )
    return in_maps


def kernel(x, mask, W, b, a_left, a_right):
    x = np.asarray(x)
    mask = np.asarray(mask)
    W = np.asarray(W).astype(np.float32)
    b = np.asarray(b).astype(np.float32)
    a_left = np.asarray(a_left).astype(np.float32)
    a_right = np.asarray(a_right).astype(np.float32)
    nc = _get_nc()
    in_maps = make_in_maps(x, mask, W, b, a_left, a_right)
    res = run_bass_kernel_spmd(nc, in_maps, core_ids=list(range(N_CORES)))
    out = np.zeros((N_NODES, K_HEADS * F_OUT), np.float32)
    for c in range(N_CORES):
        g = c // 2
        nbi = c % 2
        o = np.asarray(res.results[c]["out2"]).astype(np.float32)
        for k in range(2):
            head = 2 * g + k
            out[nbi * NB : (nbi + 1) * NB, head * F_OUT : (head + 1) * F_OUT] = o[
                k * F_OUT : (k + 1) * F_OUT, :
            ].T
    return out


if __name__ == "__main__":
    import reference as R

    inputs = {k: np.asarray(v) for k, v in R.setup_inputs().items()}
    expected = np.asarray(R.reference(**R.setup_inputs()))
    got = kernel(**inputs)
    aerr = np.abs(got - expected)
    scale = np.abs(expected).max()
    print(f"absmax err {aerr.max():.3e}  scale {scale:.3f}  rel {aerr.max() / scale:.3e}")
